# revision 13
# baseline (speedup 1.0000x reference)
"""BTT (block tensor-train) structured FC kernel for Trainium2, 8-core data parallel.

Math: y[b, (oa ob oc od)] = sum_blk sum_{r*} F0[ia,oa,ra] F1[ib,ob,rb] F2[ic,oc,rc]
F3[id,od,rd] C[rd,rc,rb,ra] x[b, (ia ib ic id)]  with all mode dims 8, ranks 2.

Host folds factors into:
  G[icid, blk, q=(rc,rd), ocod] = F2[ic,oc,rc]*F3[id,od,rd]          (stage A rhs)
  W[blk, q, iaib, oaob] = sum_{ra,rb} C[rd,rc,rb,ra] F0[ia,oa,ra] F1[ib,ob,rb]
Sharding is pure batch data-parallel (128 rows per core); the host lays tensors
out in the on-chip format.

Both stages run as full-array K=128 matmuls with block-diagonal packing of the
two batch parities (zeros in the off-diagonal blocks):
  stage A: lhsT = [p=(blockdiag icid x b_lo), m=(b_lo, iaib)] per batch pair,
           rhs = G duplicated on both partition halves, out u[(b_lo,iaib), .]
  stage B: lhsT = blockdiag W per chunk k, rhs = u, accumulate 16 chunks,
           out y[(b_lo, oaob), (h, bp, ocod)]
Schedule: pair-sized blocks pipelined A0 A1 B0 A2 B1 A3 B2 B3 so stage B's u
reads have a full block of slack behind the PSUM->SBUF copy engines. Early x
tiles stream on the sync DMA ring (first needed first), late ones on the
gpsimd ring (it starts ~1.5us later); y is written back bf16 per (pair, h)
and the very last half is split across two engines/rings to shorten the tail.
"""

import numpy as np

N_CORES = 8
B_CORE = 128

_CACHE = {}

# bp (batch-pair index 0..63) -> x tile sizes: small first tiles so stage A
# can start as soon as the first DMA lands.
_XT_SPLIT = [2, 6, 8, 8, 8, 8, 8, 8, 8]
_N_SYNC_TILES = 4  # first tiles on the sync ring; the rest on gpsimd's


def _fold_weights(cores, factors):
    cores = np.asarray(cores, dtype=np.float64)      # (4, 2,2,2,2) [rd,rc,rb,ra]
    factors = np.asarray(factors, dtype=np.float64)  # (4, 4, 8, 8, 2)
    G = np.zeros((64, 4, 4, 64), np.float64)         # [icid, blk, q, ocod]
    W = np.zeros((4, 4, 64, 64), np.float64)         # [blk, q, iaib, oaob]
    for blk in range(4):
        F0, F1, F2, F3 = (factors[blk, j] for j in range(4))
        C = cores[blk]
        G[:, blk] = np.einsum("cxr,dys->cdrsxy", F2, F3).reshape(64, 4, 64)
        w = np.einsum("srqp,axp,byq->srabxy", C, F0, F1).transpose(1, 0, 2, 3, 4, 5)
        W[blk] = w.reshape(4, 64, 64)
    g2 = G.reshape(64, 1024)                               # [icid, (blk q ocod)]
    w3 = W.reshape(16, 64, 64)                             # [k, iaib, oaob]
    g_dup = np.concatenate([g2, g2], axis=0)               # [128, 1024]
    # block-diagonal W: [p, k, b_lo, oaob]
    w_bd = np.zeros((128, 16, 2, 64), np.float64)
    for k in range(16):
        w_bd[0:64, k, 0, :] = w3[k]
        w_bd[64:128, k, 1, :] = w3[k]
    import ml_dtypes
    return (g_dup.astype(ml_dtypes.bfloat16),
            np.ascontiguousarray(w_bd.reshape(128, 2048)).astype(ml_dtypes.bfloat16))


def _build_nc():
    import concourse.mybir as mybir
    from concourse import bacc
    from concourse.tile import TileContext

    f32 = mybir.dt.float32
    bf16 = mybir.dt.bfloat16

    nc = bacc.Bacc("TRN2", target_bir_lowering=False, debug=False,
                   num_devices=N_CORES)
    # xt: host-prepared block-diagonal input [p, bp, b_lo, iaib]
    xt_d = nc.dram_tensor("xt", [128, 64, 2, 64], bf16, kind="ExternalInput")
    w_d = nc.dram_tensor("w", [128, 2048], bf16, kind="ExternalInput")
    g_d = nc.dram_tensor("g", [128, 1024], bf16, kind="ExternalInput")
    # y: partition-major [p=(b_lo, oaob), (pair, h, bp, ocod)]
    y_d = nc.dram_tensor("y", [128, 4096], bf16, kind="ExternalOutput")

    with TileContext(nc) as tc:
        with tc.tile_pool(name="const", bufs=1) as const, \
             tc.tile_pool(name="upool", bufs=2) as upool:

            g_sb = const.tile([128, 1024], bf16, tag="g_sb")
            w_sb = const.tile([128, 16, 2, 64], bf16, tag="w_sb")
            xz_tiles = []
            bp_map = []  # bp -> (tile_idx, offset)
            for j, n_bp in enumerate(_XT_SPLIT):
                xzj = const.tile([128, n_bp, 2, 64], bf16, tag=f"xz{j}")
                for o in range(n_bp):
                    bp_map.append((j, o))
                xz_tiles.append(xzj)
            # First-needed x tiles on the sync ring (it issues right after the
            # preamble); later tiles on the gpsimd ring, which starts ~1.5us
            # later but has them done long before pair 1 needs them. g halves
            # and w stream on the scalar ring in parallel.
            off = 0
            for j, n_bp in enumerate(_XT_SPLIT):
                q = nc.sync if j < _N_SYNC_TILES else nc.gpsimd
                q.dma_start(xz_tiles[j][:], xt_d[:, off:off + n_bp, :, :])
                if j == 0:
                    nc.scalar.dma_start(g_sb[:, 0:512], g_d[:, 0:512])
                elif j == 1:
                    nc.scalar.dma_start(g_sb[:, 512:1024], g_d[:, 512:1024])
                elif j == 2:
                    nc.scalar.dma_start(w_sb[:], w_d[:].rearrange(
                        "p (k bl m) -> p k bl m", k=16, bl=2))
                off += n_bp
            warm = const.tile([128, 128], bf16, tag="warm")
            nc.vector.memset(warm[:], 0.0)
            # wider warmup rhs, memset on the otherwise-idle gpsimd engine
            warm2 = const.tile([128, 512], bf16, tag="warm2")
            nc.gpsimd.memset(warm2[:], 0.0)

            # y_sb[p=(b_lo, oaob), pair, h, bp, ocod]
            y_sb = const.tile([128, 4, 2, 8, 64], bf16, tag="y_sb")

            with tc.tile_pool(name="apsum", bufs=6, space="PSUM") as apsum, \
                 tc.tile_pool(name="bpsum", bufs=2, space="PSUM") as bpsum:
                # Warmups: the first input DMA cannot land before ~9us (the
                # preamble barrier plus DMA issue+pipe latency), so keep the
                # PE busy and ramping its p-state until then.
                wps = bpsum.tile([128, 512], f32, tag="bps")
                for _ in range(5):
                    nc.tensor.matmul(wps[:], warm[:], warm2[:],
                                     start=True, stop=True)

                u_tiles = [None] * 4

                def stage_a(pair):
                    # u holds one pair group (16 batch pairs)
                    u = upool.tile([128, 16, 1024], bf16, name=f"u{pair}",
                                   tag="u")
                    u_tiles[pair] = u
                    for p2 in range(8):
                        bpe = pair * 16 + 2 * p2
                        bpo = bpe + 1
                        je, oe = bp_map[bpe]
                        jo, oo = bp_map[bpo]
                        lhs_e = xz_tiles[je][:, oe, :, :]
                        lhs_o = xz_tiles[jo][:, oo, :, :]
                        ps_el = apsum.tile([128, 512], f32, tag="aps")
                        ps_ol = apsum.tile([128, 512], f32, tag="aps")
                        ps_eh = apsum.tile([128, 512], f32, tag="aps")
                        ps_oh = apsum.tile([128, 512], f32, tag="aps")
                        nc.tensor.matmul(ps_el[:], lhs_e, g_sb[:, 0:512],
                                         start=True, stop=True)
                        nc.tensor.matmul(ps_ol[:], lhs_o, g_sb[:, 0:512],
                                         start=True, stop=True)
                        nc.tensor.matmul(ps_eh[:], lhs_e, g_sb[:, 512:1024],
                                         start=True, stop=True)
                        nc.tensor.matmul(ps_oh[:], lhs_o, g_sb[:, 512:1024],
                                         start=True, stop=True)
                        nc.scalar.copy(u[:, 2 * p2, 0:512], ps_el[:])
                        nc.vector.tensor_copy(u[:, 2 * p2 + 1, 0:512], ps_ol[:])
                        nc.vector.tensor_copy(u[:, 2 * p2, 512:1024], ps_eh[:])
                        nc.scalar.copy(u[:, 2 * p2 + 1, 512:1024], ps_oh[:])

                def stage_b(pair):
                    u = u_tiles[pair]
                    # one K=128 N=512 matmul per (h, k), accumulating over k;
                    # h outer so each half's output DMA overlaps the other
                    # half's compute. The last half's copy and DMA are split
                    # in two so the final writeback clears in half the time.
                    for h in range(2):
                        psb = bpsum.tile([128, 512], f32,
                                         name=f"psb{pair}_{h}", tag="bps")
                        for k in range(16):
                            nc.tensor.matmul(
                                psb[:],
                                w_sb[:, k, :, :],
                                u[:, h * 8:(h + 1) * 8, k * 64:(k + 1) * 64],
                                start=(k == 0), stop=(k == 15))
                        dst = y_sb[:, pair, h, :, :].rearrange(
                            "p a b -> p (a b)")
                        ybase = pair * 1024 + h * 512
                        if pair == 3 and h == 1:
                            nc.scalar.copy(dst[:, 0:256], psb[:, 0:256])
                            nc.vector.tensor_copy(dst[:, 256:512],
                                                  psb[:, 256:512])
                            nc.scalar.dma_start(y_d[:, ybase:ybase + 256],
                                                dst[:, 0:256])
                            nc.sync.dma_start(y_d[:, ybase + 256:ybase + 512],
                                              dst[:, 256:512])
                        else:
                            if h == 0:
                                nc.scalar.copy(dst, psb[:])
                            else:
                                nc.vector.tensor_copy(dst, psb[:])
                            nc.sync.dma_start(y_d[:, ybase:ybase + 512], dst)

                # software pipeline: stage B always runs one block behind the
                # matching stage A, so its u copies have a full block of slack
                # and the PE rarely stalls on the copy engines.
                stage_a(0)
                stage_a(1)
                stage_b(0)
                stage_a(2)
                stage_b(1)
                stage_a(3)
                stage_b(2)
                stage_b(3)

    nc.compile()
    return nc


def kernel(inputs, cores, factors, trace=False):
    import ml_dtypes

    x = np.ascontiguousarray(np.asarray(inputs, dtype=np.float32))
    assert x.shape == (N_CORES * B_CORE, 4096), x.shape
    g_dup, w_host = _fold_weights(cores, factors)

    from concourse.bass_utils import run_bass_kernel_spmd

    if "nc" not in _CACHE:
        _CACHE["nc"] = _build_nc()
    nc = _CACHE["nc"]

    in_maps = []
    for c in range(N_CORES):
        xc = x[c * B_CORE:(c + 1) * B_CORE].reshape(128, 64, 64)  # [b, iaib, icid]
        xz = np.zeros((128, 64, 2, 64), np.float32)
        xz[0:64, :, 0, :] = xc[0::2].transpose(2, 0, 1)   # [icid, bp, iaib]
        xz[64:128, :, 1, :] = xc[1::2].transpose(2, 0, 1)
        xt = xz.astype(ml_dtypes.bfloat16)
        in_maps.append({"xt": xt, "g": g_dup, "w": w_host})

    res = run_bass_kernel_spmd(nc, in_maps, core_ids=list(range(N_CORES)),
                               trace=trace)
    _CACHE["last_result"] = res

    out = np.empty((N_CORES * B_CORE, 4096), np.float32)
    for c in range(N_CORES):
        yp = np.asarray(res.results[c]["y"], dtype=np.float32)  # [128, 4096]
        yr = yp.reshape(2, 64, 4, 2, 8, 64)            # [b_lo, oaob, pair, h, bp, oc]
        yb = yr.transpose(2, 3, 4, 0, 1, 5).reshape(128, 4096)
        out[c * B_CORE:(c + 1) * B_CORE] = yb
    return out
